# revision 4
# baseline (speedup 1.0000x reference)
"""MoE conv-routing gate (conv3x3 -> sigmoid -> top8 -> softmax weights + load counts).

Sharding: data-parallel over batch, one image per NeuronCore (8 cores).
Device kernel per core:
  - conv3x3 as implicit GEMM: 18 K-chunks (2 c-halves x 9 taps), lhsT = x tile
    [c=128, pix=128] stationary, rhs = w [c=128, e=64] moving, fp32 PSUM accum.
  - top-8 over experts per pixel with DVE max8/max_index (jax.lax.top_k tie-break
    semantics: descending values, lowest index first on ties).
  - weights = softmax(sigmoid(top8 logits)) using Exp-table-only ACT ops.
  - per-core expert bincount via match_replace -> relu one-hot -> ones-matmul.
Host: gathers shards, sums tiny (64,) bincounts across cores, applies the
history/wrap/bias update (the "all-reduce the tiny buffers" epilogue).
"""

import sys

sys.path.insert(0, "/opt/trn_rl_repo")

import numpy as np

import concourse.bass as bass
import concourse.bacc as bacc
import concourse.mybir as mybir
import concourse.tile as tile
from concourse.bass_utils import run_bass_kernel_spmd

B, C_IN, H, W = 8, 256, 128, 128
E, K = 64, 8
NCORES = 8
ROUTE_SCALE = 1.0
UPDATE_RATE = 0.001
WRAP = 1e8

T = 16          # out rows per supertile
NSUP = H // T   # 8 supertiles
WP = W + 2      # padded row width (130)
BIG = float(2 ** 20)

_CACHE = {}


def _build_nc():
    nc = bacc.Bacc("TRN2", target_bir_lowering=False, debug=False,
                   enable_asserts=False, num_devices=NCORES)
    dt = mybir.dt
    AF = mybir.ActivationFunctionType
    AX = mybir.AxisListType

    x_d = nc.dram_tensor("x", [2, 128, (H + 2) * WP], dt.float32,
                         kind="ExternalInput").ap()
    wp_d = nc.dram_tensor("wp", [128, 18, E], dt.float32,
                          kind="ExternalInput").ap()
    wout_d = nc.dram_tensor("wout", [H * W, K], dt.float32,
                            kind="ExternalOutput").ap()
    iout_d = nc.dram_tensor("iout", [H * W, K], dt.uint32,
                            kind="ExternalOutput").ap()
    bc_d = nc.dram_tensor("bc", [1, 4 * E], dt.float32,
                          kind="ExternalOutput").ap()

    with tile.TileContext(nc) as tc:
        with (
            tc.tile_pool(name="const", bufs=1) as cpool,
            tc.tile_pool(name="xband", bufs=2) as xpool,
            tc.tile_pool(name="work", bufs=2) as spool,
            tc.tile_pool(name="psum", bufs=2, space="PSUM") as ppool,
            tc.tile_pool(name="psbc", bufs=1, space="PSUM") as pbcool,
        ):
            w_sb = cpool.tile([128, 18, E], dt.float32)
            nc.sync.dma_start(w_sb[:, :, :], wp_d[:, :, :])
            ones_sb = cpool.tile([128, 1], dt.bfloat16)
            nc.gpsimd.memset(ones_sb[:], 1.0)
            negbig_sb = cpool.tile([128, 1], dt.float32)
            nc.gpsimd.memset(negbig_sb[:], -(BIG - 1.0))

            psum_bc = pbcool.tile([1, 4 * E], dt.float32)

            for t in range(NSUP):
                r0 = T * t  # first padded row needed
                nrow = T + 2
                bands = []
                for ch in range(2):
                    bt = xpool.tile([128, nrow * WP], dt.float32,
                                    tag=f"band{ch}")
                    nc.sync.dma_start(
                        bt[:, :], x_d[ch, :, r0 * WP:(r0 + nrow) * WP])
                    bands.append(bt)

                psum = ppool.tile([128, T * E], dt.float32)
                for jj in range(T):
                    kk = 0
                    for ch in range(2):
                        for dy in range(3):
                            for dx in range(3):
                                tap = ch * 9 + dy * 3 + dx
                                nc.tensor.matmul(
                                    psum[:, jj * E:(jj + 1) * E],
                                    bands[ch][:, WP * (jj + dy) + dx:
                                              WP * (jj + dy) + dx + 128],
                                    w_sb[:, tap, :],
                                    start=(kk == 0), stop=(kk == 17),
                                )
                                kk += 1

                logits = spool.tile([128, T * E], dt.float32)
                nc.scalar.activation(logits[:, :], psum[:, :], AF.Copy)

                sel = spool.tile([128, T, K], dt.float32)
                idxs = spool.tile([128, T, K], dt.uint32)
                repl = spool.tile([128, T * E], dt.float32)
                for jj in range(T):
                    lsl = logits[:, jj * E:(jj + 1) * E]
                    nc.vector.max(sel[:, jj, :], lsl)
                    nc.vector.max_index(idxs[:, jj, :], sel[:, jj, :], lsl)
                    nc.vector.match_replace(
                        repl[:, jj * E:(jj + 1) * E], sel[:, jj, :], lsl, BIG)

                onehot = spool.tile([128, T * E], dt.bfloat16)
                nc.scalar.activation(onehot[:, :], repl[:, :], AF.Relu,
                                     bias=negbig_sb[:, :])
                for g in range(4):
                    nc.tensor.matmul(
                        psum_bc[:, :], ones_sb[:, :],
                        onehot[:, g * 4 * E:(g + 1) * 4 * E],
                        start=(t == 0 and g == 0),
                        stop=(t == NSUP - 1 and g == 3),
                        skip_group_check=True,
                    )

                # weights = softmax(sigmoid(sel)) over k, exp-table only:
                # sigmoid(l) = 1/(1+exp(-l)); softmax without max-subtract
                # (safe: sigmoid in (0,1)).
                e1 = spool.tile([128, T, K], dt.float32)
                nc.scalar.activation(e1[:, :, :], sel[:, :, :], AF.Exp,
                                     scale=-1.0)
                t1 = spool.tile([128, T, K], dt.float32)
                nc.scalar.activation(t1[:, :, :], e1[:, :, :], AF.Copy,
                                     bias=1.0)
                sg = spool.tile([128, T, K], dt.float32)
                nc.vector.reciprocal(sg[:, :, :], t1[:, :, :])
                e2 = spool.tile([128, T, K], dt.float32)
                nc.scalar.activation(e2[:, :, :], sg[:, :, :], AF.Exp)
                sums = spool.tile([128, T], dt.float32)
                nc.vector.reduce_sum(sums[:, :], e2[:, :, :], axis=AX.X)
                rr = spool.tile([128, T], dt.float32)
                nc.vector.reciprocal(rr[:, :], sums[:, :])
                wout_sb = spool.tile([128, T, K], dt.float32)
                for jj in range(T):
                    nc.scalar.activation(wout_sb[:, jj, :], e2[:, jj, :],
                                         AF.Copy, scale=rr[:, jj:jj + 1])

                dst_w = wout_d[T * 128 * t:T * 128 * (t + 1), :].rearrange(
                    "(j p) k -> p j k", p=128)
                nc.sync.dma_start(dst_w, wout_sb[:, :, :])
                dst_i = iout_d[T * 128 * t:T * 128 * (t + 1), :].rearrange(
                    "(j p) k -> p j k", p=128)
                nc.sync.dma_start(dst_i, idxs[:, :, :])

            bc_sb = cpool.tile([1, 4 * E], dt.float32)
            nc.scalar.activation(bc_sb[:, :], psum_bc[:, :], AF.Copy)
            nc.sync.dma_start(bc_d[:, :], bc_sb[:, :])

    nc.compile()
    return nc


def _numpy_reference(x, w_gate, bias, history_counts):
    xp = np.zeros((B, C_IN, H + 2, W + 2), np.float32)
    xp[:, :, 1:-1, 1:-1] = x
    logits = np.zeros((B, E, H, W), np.float32)
    for dy in range(3):
        for dx in range(3):
            logits += np.einsum("bcyx,ec->beyx",
                                xp[:, :, dy:dy + H, dx:dx + W],
                                w_gate[:, :, dy, dx], optimize=True)
    scores = 1.0 / (1.0 + np.exp(-logits))
    biased = (scores + bias[None, :, None, None]).transpose(0, 2, 3, 1)
    scores_t = scores.transpose(0, 2, 3, 1)
    idx = np.argsort(-biased, axis=-1, kind="stable")[..., :K].astype(np.int32)
    sel = np.take_along_axis(scores_t, idx, axis=-1)
    m = sel.max(axis=-1, keepdims=True)
    ex = np.exp(sel - m)
    weights_t = (ex / ex.sum(axis=-1, keepdims=True)) * ROUTE_SCALE
    weights = weights_t.transpose(0, 3, 1, 2).astype(np.float32)
    indices = idx.transpose(0, 3, 1, 2)
    counts = history_counts + np.bincount(
        idx.reshape(-1), minlength=E).astype(np.float32)
    counts = np.where(np.all(counts > WRAP), np.remainder(counts, WRAP),
                      counts).astype(np.float32)
    load_diff = counts.mean(dtype=np.float32) - counts
    new_bias = (bias + np.float32(UPDATE_RATE) * np.sign(load_diff)).astype(
        np.float32)
    return weights, indices, counts, new_bias


def kernel(x, w_gate, bias, history_counts):
    x = np.asarray(x, np.float32)
    w_gate = np.asarray(w_gate, np.float32)
    bias = np.asarray(bias, np.float32)
    history_counts = np.asarray(history_counts, np.float32)

    # Device path assumes a uniform routing bias (adding the same constant to
    # every expert leaves the top-k selection and unbiased-score weights
    # unchanged). Non-uniform bias falls back to an exact host implementation.
    if not np.all(bias == bias[0]):
        return _numpy_reference(x, w_gate, bias, history_counts)

    if "nc" not in _CACHE:
        _CACHE["nc"] = _build_nc()
    nc = _CACHE["nc"]

    # Host-side input prep (the shard/pad step of the data-parallel layout).
    xr = x.reshape(B, 2, 128, H, W)
    xpad = np.zeros((B, 2, 128, H + 2, WP), np.float32)
    xpad[:, :, :, 1:H + 1, 1:W + 1] = xr
    wr = w_gate.reshape(E, 2, 128, 3, 3)
    wp = np.ascontiguousarray(np.transpose(wr, (2, 1, 3, 4, 0))).reshape(
        128, 18, E)

    in_maps = [{"x": np.ascontiguousarray(xpad[b].reshape(2, 128, -1)),
                "wp": wp} for b in range(B)]
    res = run_bass_kernel_spmd(nc, in_maps, core_ids=list(range(NCORES)))
    outs = res.results

    weights = np.stack([outs[b]["wout"].reshape(H, W, K).transpose(2, 0, 1)
                        for b in range(B)])
    indices = np.stack([outs[b]["iout"].reshape(H, W, K).transpose(2, 0, 1)
                        for b in range(B)]).astype(np.int32)

    # tiny all-reduce of per-shard bincounts + bias update on host
    bc = np.zeros(E, np.float32)
    for b in range(B):
        bc += outs[b]["bc"].reshape(4, E).sum(axis=0)
    counts = (history_counts + bc).astype(np.float32)
    counts = np.where(np.all(counts > WRAP), np.remainder(counts, WRAP),
                      counts).astype(np.float32)
    load_diff = counts.mean(dtype=np.float32) - counts
    new_bias = (bias + np.float32(UPDATE_RATE) * np.sign(load_diff)).astype(
        np.float32)
    return weights, indices, counts, new_bias


# revision 7
# speedup vs baseline: 1.0647x; 1.0647x over previous
"""MoE conv-routing gate (conv3x3 -> sigmoid -> top8 -> softmax weights + load counts).

Sharding: data-parallel over batch, one image per NeuronCore (8 cores).
Device kernel per core:
  - conv3x3 as implicit GEMM: 18 K-chunks (2 c-halves x 9 taps), lhsT = x tile
    [c=128, pix=128] stationary, rhs = w [c=128, e=64] moving, fp32 PSUM accum.
  - top-8 over experts per pixel with DVE max8/max_index (jax.lax.top_k tie-break
    semantics: descending values, lowest index first on ties).
  - weights = softmax(sigmoid(top8 logits)) using Exp-table-only ACT ops.
  - per-core expert bincount via match_replace -> relu one-hot -> ones-matmul.
Host: gathers shards, sums tiny (64,) bincounts across cores, applies the
history/wrap/bias update (the "all-reduce the tiny buffers" epilogue).
"""

import sys

sys.path.insert(0, "/opt/trn_rl_repo")

import numpy as np

import concourse.bass as bass
import concourse.bacc as bacc
import concourse.mybir as mybir
import concourse.tile as tile
from concourse.bass_utils import run_bass_kernel_spmd

B, C_IN, H, W = 8, 256, 128, 128
E, K = 64, 8
NCORES = 8
ROUTE_SCALE = 1.0
UPDATE_RATE = 0.001
WRAP = 1e8

T = 16          # out rows per supertile
NSUP = H // T   # 8 supertiles
WP = W + 2      # padded row width (130)
BIG = float(2 ** 20)

_CACHE = {}
USE_FP16 = True


def _build_nc_fp16():
    """fp16x3 conv: x = xh + xl, w = wh + wl (fp16 hi/lo splits; all retained
    products exact in f32). logits = xh*wh + xh*wl + xl*wh accumulated in
    PSUM; dropped xl*wl term is ~2^-22 relative. 3x fewer PE cycles than the
    fp32 path (fp32 matmul = 4 cycles/row vs fp16 1 cycle/row)."""
    nc = bacc.Bacc("TRN2", target_bir_lowering=False, debug=False,
                   enable_asserts=False, num_devices=NCORES)
    dt = mybir.dt
    AF = mybir.ActivationFunctionType
    AX = mybir.AxisListType

    xh_d = nc.dram_tensor("xh", [2, 128, (H + 2) * WP], dt.float16,
                          kind="ExternalInput").ap()
    xl_d = nc.dram_tensor("xl", [2, 128, (H + 2) * WP], dt.float16,
                          kind="ExternalInput").ap()
    wh2_d = nc.dram_tensor("wh2", [128, 18, 2 * E], dt.float16,
                           kind="ExternalInput").ap()
    wout_d = nc.dram_tensor("wout", [H * W, K], dt.float32,
                            kind="ExternalOutput").ap()
    iout_d = nc.dram_tensor("iout", [H * W, K], dt.uint32,
                            kind="ExternalOutput").ap()
    bc_d = nc.dram_tensor("bc", [1, 4 * E], dt.float32,
                          kind="ExternalOutput").ap()

    with tile.TileContext(nc) as tc:
        with (
            tc.tile_pool(name="const", bufs=1) as cpool,
            tc.tile_pool(name="xband", bufs=2) as xpool,
            tc.tile_pool(name="work", bufs=2) as spool,
            tc.tile_pool(name="psum", bufs=1, space="PSUM") as ppool,
            tc.tile_pool(name="psbc", bufs=1, space="PSUM") as pbcool,
        ):
            w_sb = cpool.tile([128, 18, 2 * E], dt.float16)
            nc.sync.dma_start(w_sb[:, :, :], wh2_d[:, :, :])
            ones_sb = cpool.tile([128, 1], dt.bfloat16)
            nc.gpsimd.memset(ones_sb[:], 1.0)
            negbig_sb = cpool.tile([128, 1], dt.float32)
            nc.gpsimd.memset(negbig_sb[:], -(BIG - 1.0))

            psum_bc = pbcool.tile([1, 4 * E], dt.float32)

            for t in range(NSUP):
                r0 = T * t
                nrow = T + 2
                hbands, lbands = [], []
                for ch in range(2):
                    bh = xpool.tile([128, nrow * WP], dt.float16,
                                    tag=f"bandh{ch}")
                    nc.sync.dma_start(
                        bh[:, :], xh_d[ch, :, r0 * WP:(r0 + nrow) * WP])
                    hbands.append(bh)
                    bl = xpool.tile([128, nrow * WP], dt.float16,
                                    tag=f"bandl{ch}")
                    nc.sync.dma_start(
                        bl[:, :], xl_d[ch, :, r0 * WP:(r0 + nrow) * WP])
                    lbands.append(bl)

                psum = ppool.tile([128, T, 2 * E], dt.float32)
                for jj in range(T):
                    kk = 0
                    for ch in range(2):
                        for dy in range(3):
                            for dx in range(3):
                                tap = ch * 9 + dy * 3 + dx
                                sl = slice(WP * (jj + dy) + dx,
                                           WP * (jj + dy) + dx + 128)
                                nc.tensor.matmul(
                                    psum[:, jj, :],
                                    hbands[ch][:, sl],
                                    w_sb[:, tap, :],
                                    start=(kk == 0), stop=False,
                                    skip_group_check=True,
                                )
                                nc.tensor.matmul(
                                    psum[:, jj, 0:E],
                                    lbands[ch][:, sl],
                                    w_sb[:, tap, 0:E],
                                    start=False, stop=(kk == 17),
                                    skip_group_check=True,
                                )
                                kk += 1

                # hw: only one tensor_tensor input may come from PSUM, so
                # evacuate the hi|lo halves in two steps.
                logits = spool.tile([128, T * E], dt.float32)
                lg3 = logits.rearrange("p (t e) -> p t e", t=T)
                nc.scalar.activation(lg3[:, :, :], psum[:, :, 0:E], AF.Copy)
                nc.vector.tensor_add(lg3[:, :, :], lg3[:, :, :],
                                     psum[:, :, E:2 * E])

                sel = spool.tile([128, T, K], dt.float32)
                idxs = spool.tile([128, T, K], dt.uint32)
                repl = spool.tile([128, T * E], dt.float32)
                for jj in range(T):
                    lsl = logits[:, jj * E:(jj + 1) * E]
                    nc.vector.max(sel[:, jj, :], lsl)
                    nc.vector.max_index(idxs[:, jj, :], sel[:, jj, :], lsl)
                    nc.vector.match_replace(
                        repl[:, jj * E:(jj + 1) * E], sel[:, jj, :], lsl, BIG)

                onehot = spool.tile([128, T * E], dt.bfloat16)
                nc.scalar.activation(onehot[:, :], repl[:, :], AF.Relu,
                                     bias=negbig_sb[:, :])
                for g in range(4):
                    nc.tensor.matmul(
                        psum_bc[:, :], ones_sb[:, :],
                        onehot[:, g * 4 * E:(g + 1) * 4 * E],
                        start=(t == 0 and g == 0),
                        stop=(t == NSUP - 1 and g == 3),
                        skip_group_check=True,
                    )

                e1 = spool.tile([128, T, K], dt.float32)
                nc.scalar.activation(e1[:, :, :], sel[:, :, :], AF.Exp,
                                     scale=-1.0)
                t1 = spool.tile([128, T, K], dt.float32)
                nc.scalar.activation(t1[:, :, :], e1[:, :, :], AF.Copy,
                                     bias=1.0)
                sg = spool.tile([128, T, K], dt.float32)
                nc.vector.reciprocal(sg[:, :, :], t1[:, :, :])
                e2 = spool.tile([128, T, K], dt.float32)
                nc.scalar.activation(e2[:, :, :], sg[:, :, :], AF.Exp)
                sums = spool.tile([128, T], dt.float32)
                nc.vector.reduce_sum(sums[:, :], e2[:, :, :], axis=AX.X)
                rr = spool.tile([128, T], dt.float32)
                nc.vector.reciprocal(rr[:, :], sums[:, :])
                wout_sb = spool.tile([128, T, K], dt.float32)
                for jj in range(T):
                    nc.scalar.activation(wout_sb[:, jj, :], e2[:, jj, :],
                                         AF.Copy, scale=rr[:, jj:jj + 1])

                dst_w = wout_d[T * 128 * t:T * 128 * (t + 1), :].rearrange(
                    "(j p) k -> p j k", p=128)
                nc.sync.dma_start(dst_w, wout_sb[:, :, :])
                dst_i = iout_d[T * 128 * t:T * 128 * (t + 1), :].rearrange(
                    "(j p) k -> p j k", p=128)
                nc.sync.dma_start(dst_i, idxs[:, :, :])

            bc_sb = cpool.tile([1, 4 * E], dt.float32)
            nc.scalar.activation(bc_sb[:, :], psum_bc[:, :], AF.Copy)
            nc.sync.dma_start(bc_d[:, :], bc_sb[:, :])

    nc.compile()
    return nc


def _build_nc():
    nc = bacc.Bacc("TRN2", target_bir_lowering=False, debug=False,
                   enable_asserts=False, num_devices=NCORES)
    dt = mybir.dt
    AF = mybir.ActivationFunctionType
    AX = mybir.AxisListType

    x_d = nc.dram_tensor("x", [2, 128, (H + 2) * WP], dt.float32,
                         kind="ExternalInput").ap()
    wp_d = nc.dram_tensor("wp", [128, 18, E], dt.float32,
                          kind="ExternalInput").ap()
    wout_d = nc.dram_tensor("wout", [H * W, K], dt.float32,
                            kind="ExternalOutput").ap()
    iout_d = nc.dram_tensor("iout", [H * W, K], dt.uint32,
                            kind="ExternalOutput").ap()
    bc_d = nc.dram_tensor("bc", [1, 4 * E], dt.float32,
                          kind="ExternalOutput").ap()

    with tile.TileContext(nc) as tc:
        with (
            tc.tile_pool(name="const", bufs=1) as cpool,
            tc.tile_pool(name="xband", bufs=2) as xpool,
            tc.tile_pool(name="work", bufs=2) as spool,
            tc.tile_pool(name="psum", bufs=2, space="PSUM") as ppool,
            tc.tile_pool(name="psbc", bufs=1, space="PSUM") as pbcool,
        ):
            w_sb = cpool.tile([128, 18, E], dt.float32)
            nc.sync.dma_start(w_sb[:, :, :], wp_d[:, :, :])
            ones_sb = cpool.tile([128, 1], dt.bfloat16)
            nc.gpsimd.memset(ones_sb[:], 1.0)
            negbig_sb = cpool.tile([128, 1], dt.float32)
            nc.gpsimd.memset(negbig_sb[:], -(BIG - 1.0))

            psum_bc = pbcool.tile([1, 4 * E], dt.float32)

            for t in range(NSUP):
                r0 = T * t  # first padded row needed
                nrow = T + 2
                bands = []
                for ch in range(2):
                    bt = xpool.tile([128, nrow * WP], dt.float32,
                                    tag=f"band{ch}")
                    nc.sync.dma_start(
                        bt[:, :], x_d[ch, :, r0 * WP:(r0 + nrow) * WP])
                    bands.append(bt)

                psum = ppool.tile([128, T * E], dt.float32)
                for jj in range(T):
                    kk = 0
                    for ch in range(2):
                        for dy in range(3):
                            for dx in range(3):
                                tap = ch * 9 + dy * 3 + dx
                                nc.tensor.matmul(
                                    psum[:, jj * E:(jj + 1) * E],
                                    bands[ch][:, WP * (jj + dy) + dx:
                                              WP * (jj + dy) + dx + 128],
                                    w_sb[:, tap, :],
                                    start=(kk == 0), stop=(kk == 17),
                                )
                                kk += 1

                logits = spool.tile([128, T * E], dt.float32)
                nc.scalar.activation(logits[:, :], psum[:, :], AF.Copy)

                sel = spool.tile([128, T, K], dt.float32)
                idxs = spool.tile([128, T, K], dt.uint32)
                repl = spool.tile([128, T * E], dt.float32)
                for jj in range(T):
                    lsl = logits[:, jj * E:(jj + 1) * E]
                    nc.vector.max(sel[:, jj, :], lsl)
                    nc.vector.max_index(idxs[:, jj, :], sel[:, jj, :], lsl)
                    nc.vector.match_replace(
                        repl[:, jj * E:(jj + 1) * E], sel[:, jj, :], lsl, BIG)

                onehot = spool.tile([128, T * E], dt.bfloat16)
                nc.scalar.activation(onehot[:, :], repl[:, :], AF.Relu,
                                     bias=negbig_sb[:, :])
                for g in range(4):
                    nc.tensor.matmul(
                        psum_bc[:, :], ones_sb[:, :],
                        onehot[:, g * 4 * E:(g + 1) * 4 * E],
                        start=(t == 0 and g == 0),
                        stop=(t == NSUP - 1 and g == 3),
                        skip_group_check=True,
                    )

                # weights = softmax(sigmoid(sel)) over k, exp-table only:
                # sigmoid(l) = 1/(1+exp(-l)); softmax without max-subtract
                # (safe: sigmoid in (0,1)).
                e1 = spool.tile([128, T, K], dt.float32)
                nc.scalar.activation(e1[:, :, :], sel[:, :, :], AF.Exp,
                                     scale=-1.0)
                t1 = spool.tile([128, T, K], dt.float32)
                nc.scalar.activation(t1[:, :, :], e1[:, :, :], AF.Copy,
                                     bias=1.0)
                sg = spool.tile([128, T, K], dt.float32)
                nc.vector.reciprocal(sg[:, :, :], t1[:, :, :])
                e2 = spool.tile([128, T, K], dt.float32)
                nc.scalar.activation(e2[:, :, :], sg[:, :, :], AF.Exp)
                sums = spool.tile([128, T], dt.float32)
                nc.vector.reduce_sum(sums[:, :], e2[:, :, :], axis=AX.X)
                rr = spool.tile([128, T], dt.float32)
                nc.vector.reciprocal(rr[:, :], sums[:, :])
                wout_sb = spool.tile([128, T, K], dt.float32)
                for jj in range(T):
                    nc.scalar.activation(wout_sb[:, jj, :], e2[:, jj, :],
                                         AF.Copy, scale=rr[:, jj:jj + 1])

                dst_w = wout_d[T * 128 * t:T * 128 * (t + 1), :].rearrange(
                    "(j p) k -> p j k", p=128)
                nc.sync.dma_start(dst_w, wout_sb[:, :, :])
                dst_i = iout_d[T * 128 * t:T * 128 * (t + 1), :].rearrange(
                    "(j p) k -> p j k", p=128)
                nc.sync.dma_start(dst_i, idxs[:, :, :])

            bc_sb = cpool.tile([1, 4 * E], dt.float32)
            nc.scalar.activation(bc_sb[:, :], psum_bc[:, :], AF.Copy)
            nc.sync.dma_start(bc_d[:, :], bc_sb[:, :])

    nc.compile()
    return nc


def _numpy_reference(x, w_gate, bias, history_counts):
    xp = np.zeros((B, C_IN, H + 2, W + 2), np.float32)
    xp[:, :, 1:-1, 1:-1] = x
    logits = np.zeros((B, E, H, W), np.float32)
    for dy in range(3):
        for dx in range(3):
            logits += np.einsum("bcyx,ec->beyx",
                                xp[:, :, dy:dy + H, dx:dx + W],
                                w_gate[:, :, dy, dx], optimize=True)
    scores = 1.0 / (1.0 + np.exp(-logits))
    biased = (scores + bias[None, :, None, None]).transpose(0, 2, 3, 1)
    scores_t = scores.transpose(0, 2, 3, 1)
    idx = np.argsort(-biased, axis=-1, kind="stable")[..., :K].astype(np.int32)
    sel = np.take_along_axis(scores_t, idx, axis=-1)
    m = sel.max(axis=-1, keepdims=True)
    ex = np.exp(sel - m)
    weights_t = (ex / ex.sum(axis=-1, keepdims=True)) * ROUTE_SCALE
    weights = weights_t.transpose(0, 3, 1, 2).astype(np.float32)
    indices = idx.transpose(0, 3, 1, 2)
    counts = history_counts + np.bincount(
        idx.reshape(-1), minlength=E).astype(np.float32)
    counts = np.where(np.all(counts > WRAP), np.remainder(counts, WRAP),
                      counts).astype(np.float32)
    load_diff = counts.mean(dtype=np.float32) - counts
    new_bias = (bias + np.float32(UPDATE_RATE) * np.sign(load_diff)).astype(
        np.float32)
    return weights, indices, counts, new_bias


def kernel(x, w_gate, bias, history_counts):
    x = np.asarray(x, np.float32)
    w_gate = np.asarray(w_gate, np.float32)
    bias = np.asarray(bias, np.float32)
    history_counts = np.asarray(history_counts, np.float32)

    # Device path assumes a uniform routing bias (adding the same constant to
    # every expert leaves the top-k selection and unbiased-score weights
    # unchanged). Non-uniform bias falls back to an exact host implementation.
    if not np.all(bias == bias[0]):
        return _numpy_reference(x, w_gate, bias, history_counts)

    key = "nc16" if USE_FP16 else "nc"
    if key not in _CACHE:
        _CACHE[key] = _build_nc_fp16() if USE_FP16 else _build_nc()
    nc = _CACHE[key]

    # Host-side input prep (the shard/pad step of the data-parallel layout).
    xr = x.reshape(B, 2, 128, H, W)
    wr = w_gate.reshape(E, 2, 128, 3, 3)
    wp = np.ascontiguousarray(np.transpose(wr, (2, 1, 3, 4, 0))).reshape(
        128, 18, E)

    if USE_FP16:
        xh = xr.astype(np.float16)
        xl = (xr - xh.astype(np.float32)).astype(np.float16)
        ph = np.zeros((B, 2, 128, H + 2, WP), np.float16)
        pl = np.zeros((B, 2, 128, H + 2, WP), np.float16)
        ph[:, :, :, 1:H + 1, 1:W + 1] = xh
        pl[:, :, :, 1:H + 1, 1:W + 1] = xl
        wh = wp.astype(np.float16)
        wl = (wp - wh.astype(np.float32)).astype(np.float16)
        wh2 = np.ascontiguousarray(np.concatenate([wh, wl], axis=2))
        in_maps = [{"xh": np.ascontiguousarray(ph[b].reshape(2, 128, -1)),
                    "xl": np.ascontiguousarray(pl[b].reshape(2, 128, -1)),
                    "wh2": wh2} for b in range(B)]
    else:
        xpad = np.zeros((B, 2, 128, H + 2, WP), np.float32)
        xpad[:, :, :, 1:H + 1, 1:W + 1] = xr
        in_maps = [{"x": np.ascontiguousarray(xpad[b].reshape(2, 128, -1)),
                    "wp": wp} for b in range(B)]
    res = run_bass_kernel_spmd(nc, in_maps, core_ids=list(range(NCORES)))
    outs = res.results

    weights = np.stack([outs[b]["wout"].reshape(H, W, K).transpose(2, 0, 1)
                        for b in range(B)])
    indices = np.stack([outs[b]["iout"].reshape(H, W, K).transpose(2, 0, 1)
                        for b in range(B)]).astype(np.int32)

    # tiny all-reduce of per-shard bincounts + bias update on host
    bc = np.zeros(E, np.float32)
    for b in range(B):
        bc += outs[b]["bc"].reshape(4, E).sum(axis=0)
    counts = (history_counts + bc).astype(np.float32)
    counts = np.where(np.all(counts > WRAP), np.remainder(counts, WRAP),
                      counts).astype(np.float32)
    load_diff = counts.mean(dtype=np.float32) - counts
    new_bias = (bias + np.float32(UPDATE_RATE) * np.sign(load_diff)).astype(
        np.float32)
    return weights, indices, counts, new_bias


# revision 10
# speedup vs baseline: 41375.3971x; 38859.4891x over previous
"""MoE conv-routing gate (conv3x3 -> sigmoid -> top8 -> softmax weights + load counts).

Sharding: data-parallel over batch, one image per NeuronCore (8 cores).
Device kernel per core:
  - conv3x3 as implicit GEMM: 18 K-chunks (2 c-halves x 9 taps), lhsT = x tile
    [c=128, pix=128] stationary, rhs = w [c=128, e=64] moving, fp32 PSUM accum.
  - top-8 over experts per pixel with DVE max8/max_index (jax.lax.top_k tie-break
    semantics: descending values, lowest index first on ties).
  - weights = softmax(sigmoid(top8 logits)) using Exp-table-only ACT ops.
  - per-core expert bincount via match_replace -> relu one-hot -> ones-matmul.
Host: gathers shards, sums tiny (64,) bincounts across cores, applies the
history/wrap/bias update (the "all-reduce the tiny buffers" epilogue).
"""

import sys

sys.path.insert(0, "/opt/trn_rl_repo")

import numpy as np

import concourse.bass as bass
import concourse.bacc as bacc
import concourse.mybir as mybir
import concourse.tile as tile
from concourse.bass_utils import run_bass_kernel_spmd

B, C_IN, H, W = 8, 256, 128, 128
E, K = 64, 8
NCORES = 8
ROUTE_SCALE = 1.0
UPDATE_RATE = 0.001
WRAP = 1e8

T = 16          # out rows per supertile
NSUP = H // T   # 8 supertiles
WP = W + 2      # padded row width (130)
BIG = float(2 ** 20)

_CACHE = {}
USE_FP16 = True


def _build_nc_fp16():
    """fp16x3 conv: x = xh + xl, w = wh + wl (fp16 hi/lo splits; all retained
    products exact in f32). logits = xh*wh + xh*wl + xl*wh accumulated in
    PSUM; dropped xl*wl term is ~2^-22 relative. 3x fewer PE cycles than the
    fp32 path (fp32 matmul = 4 cycles/row vs fp16 1 cycle/row)."""
    T = 8       # smaller supertile: 2-bank PSUM tiles allow double-buffering
    NSUP = H // T
    nc = bacc.Bacc("TRN2", target_bir_lowering=False, debug=False,
                   enable_asserts=False, num_devices=NCORES)
    dt = mybir.dt
    AF = mybir.ActivationFunctionType
    AX = mybir.AxisListType

    xh_d = nc.dram_tensor("xh", [2, 128, (H + 2) * WP], dt.float16,
                          kind="ExternalInput").ap()
    xl_d = nc.dram_tensor("xl", [2, 128, (H + 2) * WP], dt.float16,
                          kind="ExternalInput").ap()
    wh2_d = nc.dram_tensor("wh2", [128, 18, 2 * E], dt.float16,
                           kind="ExternalInput").ap()
    wout_d = nc.dram_tensor("wout", [H * W, K], dt.float32,
                            kind="ExternalOutput").ap()
    iout_d = nc.dram_tensor("iout", [H * W, K], dt.uint32,
                            kind="ExternalOutput").ap()
    bc_d = nc.dram_tensor("bc", [1, 4 * E], dt.float32,
                          kind="ExternalOutput").ap()

    with tile.TileContext(nc) as tc:
        with (
            tc.tile_pool(name="const", bufs=1) as cpool,
            tc.tile_pool(name="xband", bufs=2) as xpool,
            tc.tile_pool(name="work", bufs=2) as spool,
            tc.tile_pool(name="psum", bufs=2, space="PSUM") as ppool,
            tc.tile_pool(name="psbc", bufs=1, space="PSUM") as pbcool,
        ):
            w_sb = cpool.tile([128, 18, 2 * E], dt.float16)
            nc.sync.dma_start(w_sb[:, :, :], wh2_d[:, :, :])
            ones_sb = cpool.tile([128, 1], dt.bfloat16)
            nc.gpsimd.memset(ones_sb[:], 1.0)
            negbig_sb = cpool.tile([128, 1], dt.float32)
            nc.gpsimd.memset(negbig_sb[:], -(BIG - 1.0))

            psum_bc = pbcool.tile([1, 4 * E], dt.float32)

            for t in range(NSUP):
                r0 = T * t
                nrow = T + 2
                hbands, lbands = [], []
                for ch in range(2):
                    bh = xpool.tile([128, nrow * WP], dt.float16,
                                    tag=f"bandh{ch}")
                    nc.sync.dma_start(
                        bh[:, :], xh_d[ch, :, r0 * WP:(r0 + nrow) * WP])
                    hbands.append(bh)
                    bl = xpool.tile([128, nrow * WP], dt.float16,
                                    tag=f"bandl{ch}")
                    nc.sync.dma_start(
                        bl[:, :], xl_d[ch, :, r0 * WP:(r0 + nrow) * WP])
                    lbands.append(bl)

                psum = ppool.tile([128, T, 2 * E], dt.float32)
                for jj in range(T):
                    kk = 0
                    for ch in range(2):
                        for dy in range(3):
                            for dx in range(3):
                                tap = ch * 9 + dy * 3 + dx
                                sl = slice(WP * (jj + dy) + dx,
                                           WP * (jj + dy) + dx + 128)
                                nc.tensor.matmul(
                                    psum[:, jj, :],
                                    hbands[ch][:, sl],
                                    w_sb[:, tap, :],
                                    start=(kk == 0), stop=False,
                                    skip_group_check=True,
                                )
                                nc.tensor.matmul(
                                    psum[:, jj, 0:E],
                                    lbands[ch][:, sl],
                                    w_sb[:, tap, 0:E],
                                    start=False, stop=(kk == 17),
                                    skip_group_check=True,
                                )
                                kk += 1

                # hw: only one tensor_tensor input may come from PSUM, so
                # evacuate the hi|lo halves in two steps.
                logits = spool.tile([128, T * E], dt.float32)
                lg3 = logits.rearrange("p (t e) -> p t e", t=T)
                nc.scalar.activation(lg3[:, :, :], psum[:, :, 0:E], AF.Copy)
                nc.vector.tensor_add(lg3[:, :, :], lg3[:, :, :],
                                     psum[:, :, E:2 * E])

                sel = spool.tile([128, T, K], dt.float32)
                idxs = spool.tile([128, T, K], dt.uint32)
                repl = spool.tile([128, T * E], dt.float32)
                for jj in range(T):
                    lsl = logits[:, jj * E:(jj + 1) * E]
                    nc.vector.max(sel[:, jj, :], lsl)
                    nc.vector.max_index(idxs[:, jj, :], sel[:, jj, :], lsl)
                    nc.vector.match_replace(
                        repl[:, jj * E:(jj + 1) * E], sel[:, jj, :], lsl, BIG)

                onehot = spool.tile([128, T * E], dt.bfloat16)
                nc.scalar.activation(onehot[:, :], repl[:, :], AF.Relu,
                                     bias=negbig_sb[:, :])
                ng = T // 4
                for g in range(ng):
                    nc.tensor.matmul(
                        psum_bc[:, :], ones_sb[:, :],
                        onehot[:, g * 4 * E:(g + 1) * 4 * E],
                        start=(t == 0 and g == 0),
                        stop=(t == NSUP - 1 and g == ng - 1),
                        skip_group_check=True,
                    )

                e1 = spool.tile([128, T, K], dt.float32)
                nc.scalar.activation(e1[:, :, :], sel[:, :, :], AF.Exp,
                                     scale=-1.0)
                t1 = spool.tile([128, T, K], dt.float32)
                nc.scalar.activation(t1[:, :, :], e1[:, :, :], AF.Copy,
                                     bias=1.0)
                sg = spool.tile([128, T, K], dt.float32)
                nc.vector.reciprocal(sg[:, :, :], t1[:, :, :])
                e2 = spool.tile([128, T, K], dt.float32)
                nc.scalar.activation(e2[:, :, :], sg[:, :, :], AF.Exp)
                sums = spool.tile([128, T], dt.float32)
                nc.vector.reduce_sum(sums[:, :], e2[:, :, :], axis=AX.X)
                rr = spool.tile([128, T], dt.float32)
                nc.vector.reciprocal(rr[:, :], sums[:, :])
                wout_sb = spool.tile([128, T, K], dt.float32)
                for jj in range(T):
                    nc.scalar.activation(wout_sb[:, jj, :], e2[:, jj, :],
                                         AF.Copy, scale=rr[:, jj:jj + 1])

                dst_w = wout_d[T * 128 * t:T * 128 * (t + 1), :].rearrange(
                    "(j p) k -> p j k", p=128)
                nc.sync.dma_start(dst_w, wout_sb[:, :, :])
                dst_i = iout_d[T * 128 * t:T * 128 * (t + 1), :].rearrange(
                    "(j p) k -> p j k", p=128)
                nc.sync.dma_start(dst_i, idxs[:, :, :])

            bc_sb = cpool.tile([1, 4 * E], dt.float32)
            nc.scalar.activation(bc_sb[:, :], psum_bc[:, :], AF.Copy)
            nc.sync.dma_start(bc_d[:, :], bc_sb[:, :])

    nc.compile()
    return nc


def _build_nc():
    nc = bacc.Bacc("TRN2", target_bir_lowering=False, debug=False,
                   enable_asserts=False, num_devices=NCORES)
    dt = mybir.dt
    AF = mybir.ActivationFunctionType
    AX = mybir.AxisListType

    x_d = nc.dram_tensor("x", [2, 128, (H + 2) * WP], dt.float32,
                         kind="ExternalInput").ap()
    wp_d = nc.dram_tensor("wp", [128, 18, E], dt.float32,
                          kind="ExternalInput").ap()
    wout_d = nc.dram_tensor("wout", [H * W, K], dt.float32,
                            kind="ExternalOutput").ap()
    iout_d = nc.dram_tensor("iout", [H * W, K], dt.uint32,
                            kind="ExternalOutput").ap()
    bc_d = nc.dram_tensor("bc", [1, 4 * E], dt.float32,
                          kind="ExternalOutput").ap()

    with tile.TileContext(nc) as tc:
        with (
            tc.tile_pool(name="const", bufs=1) as cpool,
            tc.tile_pool(name="xband", bufs=2) as xpool,
            tc.tile_pool(name="work", bufs=2) as spool,
            tc.tile_pool(name="psum", bufs=2, space="PSUM") as ppool,
            tc.tile_pool(name="psbc", bufs=1, space="PSUM") as pbcool,
        ):
            w_sb = cpool.tile([128, 18, E], dt.float32)
            nc.sync.dma_start(w_sb[:, :, :], wp_d[:, :, :])
            ones_sb = cpool.tile([128, 1], dt.bfloat16)
            nc.gpsimd.memset(ones_sb[:], 1.0)
            negbig_sb = cpool.tile([128, 1], dt.float32)
            nc.gpsimd.memset(negbig_sb[:], -(BIG - 1.0))

            psum_bc = pbcool.tile([1, 4 * E], dt.float32)

            for t in range(NSUP):
                r0 = T * t  # first padded row needed
                nrow = T + 2
                bands = []
                for ch in range(2):
                    bt = xpool.tile([128, nrow * WP], dt.float32,
                                    tag=f"band{ch}")
                    nc.sync.dma_start(
                        bt[:, :], x_d[ch, :, r0 * WP:(r0 + nrow) * WP])
                    bands.append(bt)

                psum = ppool.tile([128, T * E], dt.float32)
                for jj in range(T):
                    kk = 0
                    for ch in range(2):
                        for dy in range(3):
                            for dx in range(3):
                                tap = ch * 9 + dy * 3 + dx
                                nc.tensor.matmul(
                                    psum[:, jj * E:(jj + 1) * E],
                                    bands[ch][:, WP * (jj + dy) + dx:
                                              WP * (jj + dy) + dx + 128],
                                    w_sb[:, tap, :],
                                    start=(kk == 0), stop=(kk == 17),
                                )
                                kk += 1

                logits = spool.tile([128, T * E], dt.float32)
                nc.scalar.activation(logits[:, :], psum[:, :], AF.Copy)

                sel = spool.tile([128, T, K], dt.float32)
                idxs = spool.tile([128, T, K], dt.uint32)
                repl = spool.tile([128, T * E], dt.float32)
                for jj in range(T):
                    lsl = logits[:, jj * E:(jj + 1) * E]
                    nc.vector.max(sel[:, jj, :], lsl)
                    nc.vector.max_index(idxs[:, jj, :], sel[:, jj, :], lsl)
                    nc.vector.match_replace(
                        repl[:, jj * E:(jj + 1) * E], sel[:, jj, :], lsl, BIG)

                onehot = spool.tile([128, T * E], dt.bfloat16)
                nc.scalar.activation(onehot[:, :], repl[:, :], AF.Relu,
                                     bias=negbig_sb[:, :])
                for g in range(4):
                    nc.tensor.matmul(
                        psum_bc[:, :], ones_sb[:, :],
                        onehot[:, g * 4 * E:(g + 1) * 4 * E],
                        start=(t == 0 and g == 0),
                        stop=(t == NSUP - 1 and g == 3),
                        skip_group_check=True,
                    )

                # weights = softmax(sigmoid(sel)) over k, exp-table only:
                # sigmoid(l) = 1/(1+exp(-l)); softmax without max-subtract
                # (safe: sigmoid in (0,1)).
                e1 = spool.tile([128, T, K], dt.float32)
                nc.scalar.activation(e1[:, :, :], sel[:, :, :], AF.Exp,
                                     scale=-1.0)
                t1 = spool.tile([128, T, K], dt.float32)
                nc.scalar.activation(t1[:, :, :], e1[:, :, :], AF.Copy,
                                     bias=1.0)
                sg = spool.tile([128, T, K], dt.float32)
                nc.vector.reciprocal(sg[:, :, :], t1[:, :, :])
                e2 = spool.tile([128, T, K], dt.float32)
                nc.scalar.activation(e2[:, :, :], sg[:, :, :], AF.Exp)
                sums = spool.tile([128, T], dt.float32)
                nc.vector.reduce_sum(sums[:, :], e2[:, :, :], axis=AX.X)
                rr = spool.tile([128, T], dt.float32)
                nc.vector.reciprocal(rr[:, :], sums[:, :])
                wout_sb = spool.tile([128, T, K], dt.float32)
                for jj in range(T):
                    nc.scalar.activation(wout_sb[:, jj, :], e2[:, jj, :],
                                         AF.Copy, scale=rr[:, jj:jj + 1])

                dst_w = wout_d[T * 128 * t:T * 128 * (t + 1), :].rearrange(
                    "(j p) k -> p j k", p=128)
                nc.sync.dma_start(dst_w, wout_sb[:, :, :])
                dst_i = iout_d[T * 128 * t:T * 128 * (t + 1), :].rearrange(
                    "(j p) k -> p j k", p=128)
                nc.sync.dma_start(dst_i, idxs[:, :, :])

            bc_sb = cpool.tile([1, 4 * E], dt.float32)
            nc.scalar.activation(bc_sb[:, :], psum_bc[:, :], AF.Copy)
            nc.sync.dma_start(bc_d[:, :], bc_sb[:, :])

    nc.compile()
    return nc


def _numpy_reference(x, w_gate, bias, history_counts):
    xp = np.zeros((B, C_IN, H + 2, W + 2), np.float32)
    xp[:, :, 1:-1, 1:-1] = x
    logits = np.zeros((B, E, H, W), np.float32)
    for dy in range(3):
        for dx in range(3):
            logits += np.einsum("bcyx,ec->beyx",
                                xp[:, :, dy:dy + H, dx:dx + W],
                                w_gate[:, :, dy, dx], optimize=True)
    scores = 1.0 / (1.0 + np.exp(-logits))
    biased = (scores + bias[None, :, None, None]).transpose(0, 2, 3, 1)
    scores_t = scores.transpose(0, 2, 3, 1)
    idx = np.argsort(-biased, axis=-1, kind="stable")[..., :K].astype(np.int32)
    sel = np.take_along_axis(scores_t, idx, axis=-1)
    m = sel.max(axis=-1, keepdims=True)
    ex = np.exp(sel - m)
    weights_t = (ex / ex.sum(axis=-1, keepdims=True)) * ROUTE_SCALE
    weights = weights_t.transpose(0, 3, 1, 2).astype(np.float32)
    indices = idx.transpose(0, 3, 1, 2)
    counts = history_counts + np.bincount(
        idx.reshape(-1), minlength=E).astype(np.float32)
    counts = np.where(np.all(counts > WRAP), np.remainder(counts, WRAP),
                      counts).astype(np.float32)
    load_diff = counts.mean(dtype=np.float32) - counts
    new_bias = (bias + np.float32(UPDATE_RATE) * np.sign(load_diff)).astype(
        np.float32)
    return weights, indices, counts, new_bias


def kernel(x, w_gate, bias, history_counts):
    x = np.asarray(x, np.float32)
    w_gate = np.asarray(w_gate, np.float32)
    bias = np.asarray(bias, np.float32)
    history_counts = np.asarray(history_counts, np.float32)

    # Device path assumes a uniform routing bias (adding the same constant to
    # every expert leaves the top-k selection and unbiased-score weights
    # unchanged). Non-uniform bias falls back to an exact host implementation.
    if not np.all(bias == bias[0]):
        return _numpy_reference(x, w_gate, bias, history_counts)

    key = "nc16" if USE_FP16 else "nc"
    if key not in _CACHE:
        _CACHE[key] = _build_nc_fp16() if USE_FP16 else _build_nc()
    nc = _CACHE[key]

    # Host-side input prep (the shard/pad step of the data-parallel layout).
    xr = x.reshape(B, 2, 128, H, W)
    wr = w_gate.reshape(E, 2, 128, 3, 3)
    wp = np.ascontiguousarray(np.transpose(wr, (2, 1, 3, 4, 0))).reshape(
        128, 18, E)

    if USE_FP16:
        xh = xr.astype(np.float16)
        xl = (xr - xh.astype(np.float32)).astype(np.float16)
        ph = np.zeros((B, 2, 128, H + 2, WP), np.float16)
        pl = np.zeros((B, 2, 128, H + 2, WP), np.float16)
        ph[:, :, :, 1:H + 1, 1:W + 1] = xh
        pl[:, :, :, 1:H + 1, 1:W + 1] = xl
        wh = wp.astype(np.float16)
        wl = (wp - wh.astype(np.float32)).astype(np.float16)
        wh2 = np.ascontiguousarray(np.concatenate([wh, wl], axis=2))
        in_maps = [{"xh": np.ascontiguousarray(ph[b].reshape(2, 128, -1)),
                    "xl": np.ascontiguousarray(pl[b].reshape(2, 128, -1)),
                    "wh2": wh2} for b in range(B)]
    else:
        xpad = np.zeros((B, 2, 128, H + 2, WP), np.float32)
        xpad[:, :, :, 1:H + 1, 1:W + 1] = xr
        in_maps = [{"x": np.ascontiguousarray(xpad[b].reshape(2, 128, -1)),
                    "wp": wp} for b in range(B)]
    res = run_bass_kernel_spmd(nc, in_maps, core_ids=list(range(NCORES)))
    outs = res.results

    weights = np.stack([outs[b]["wout"].reshape(H, W, K).transpose(2, 0, 1)
                        for b in range(B)])
    indices = np.stack([outs[b]["iout"].reshape(H, W, K).transpose(2, 0, 1)
                        for b in range(B)]).astype(np.int32)

    # tiny all-reduce of per-shard bincounts + bias update on host
    bc = np.zeros(E, np.float32)
    for b in range(B):
        bc += outs[b]["bc"].reshape(4, E).sum(axis=0)
    counts = (history_counts + bc).astype(np.float32)
    counts = np.where(np.all(counts > WRAP), np.remainder(counts, WRAP),
                      counts).astype(np.float32)
    load_diff = counts.mean(dtype=np.float32) - counts
    new_bias = (bias + np.float32(UPDATE_RATE) * np.sign(load_diff)).astype(
        np.float32)
    return weights, indices, counts, new_bias
